# revision 1
# baseline (speedup 1.0000x reference)
"""Trainium2 Bass kernel for nn_BandwidthPredictorNNHall.

Math: for each batch b (8 of them, one per NeuronCore) with particles
x [n=1024, d=4]:
    pilot_d = 1.0592 * std(x_d, ddof=1) * n^(-1/8)
    q = x / pilot,   K_ij = exp(-0.5 * |q_i - q_j|^2)
    s2_d = sum_ij K_ij ((q_jd - q_id)^2 - 1)
    s3_d = sum_ij K_ij (dx^3 - 3 dx)  == 0 exactly (odd under i<->j swap),
           so bandwidth2 is fp-cancellation noise in the reference
           (|bw2/bw1| ~ 6e-9) and is treated as 0.
With Mp = [1, p_1..p_4, p_1^2..p_4^2] (n x 9, RAW particle units), every sum
needed for s2 is an entry of V = Mp^T K Mp after a host-side 1/pilot^2
rescale:
    s2_d = ((V[0,5+d] + V[5+d,0] - 2 V[1+d,1+d]) / pilot_d^2 - V[0,0]) / sqrt(2pi)
The device computes V (9x9) and var (4) per batch; the host applies the
final ~30 scalar flops per batch.

Device pipeline per core (engine-balance driven; ScalarE's 1M exps are the
floor, everything else hides behind or around them):
  - One input DMA (each dma_start costs ~0.6us of queue time plus ~1.5us
    latency): a 3D-strided load mstatall [128, 8(tile), 4] in particle-major
    layout. The feature-major Gram operands are built from it with 8 small
    PE transposes instead of a second (slow, 4-byte-run) strided DMA.
  - sum(p) and sum(p^2) accumulate on the PE as two sequential matmul
    groups against a ones vector; tiny PE transposes move the results into
    row form. var/pinv2 = 1/(FACT^2 var) needs only a reciprocal -- no
    sqrt, so ScalarE runs just {Exp, Copy}: one activation-table set, one
    LoadActFuncSet, and it overlaps the DMA latency.
  - G_ij = q_i . q_j is computed as sum_d (p_id/pilot_d^2) p_jd (float32r
    streams at 1 cycle/row; even bf16-coarse rounding would move the final
    output < 1.5e-4, far below the reference's own fp32 noise).
  - K'' = exp(G - r_i/2): one [128,1024] ScalarE activation per row tile
    with per-partition bias, reading a 2-bank PSUM tile. r_i comes from a
    multiply + negated reduce against a PE-broadcast 0.5/pilot^2 row.
  - K'' is the true K column-scaled by c_j = e^{+r_j/2}; the scale is
    constant per column so it factors through P = K M and is cancelled
    exactly in stage V by MX = Mp . e^{-r/2}:
        PT  = Mp^T K''   (9-column weight loads, f32r stream, two PSUM
                          accumulation groups that chase the exp stream)
        P'' = PT^T per 128-col block (8 small PE transposes, one PSUM bank)
        V   = MX^T P'' = Mp^T K Mp
  - K symmetry makes the stored K'' row-tiles serve both orientations, so
    the [n,n] matrix is never transposed.
"""

import sys

sys.path.insert(0, "/opt/trn_rl_repo")

import numpy as np

_B, _N, _D = 8, 1024, 4
_P = 128
_NT = _N // _P  # 8 row tiles
_NM = 1 + 2 * _D  # 9 basis columns: [1, p, p^2]
_INV_SQRT_2PI = 1.0 / np.sqrt(2.0 * np.pi)
_RK = 0.282095
_FACT = 1.0592 * float(_N) ** (-1.0 / (4 + _D))

_NC = None  # compiled Bass module cache


def _build_kernel():
    import concourse.bass as bass  # noqa: F401
    import concourse.tile as tile
    from concourse import bacc, mybir
    from concourse.masks import make_identity

    f32 = mybir.dt.float32
    fr = mybir.dt.float32r
    Act = mybir.ActivationFunctionType
    Alu = mybir.AluOpType
    Ax = mybir.AxisListType

    nc = bacc.Bacc("TRN2", target_bir_lowering=False, debug=False, num_devices=_B)
    p_in = nc.dram_tensor("p", [_N, _D], f32, kind="ExternalInput")
    v_out = nc.dram_tensor("vout", [_NM, _NM], f32, kind="ExternalOutput")
    var_out = nc.dram_tensor("varout", [_D, 1], f32, kind="ExternalOutput")

    with tile.TileContext(nc) as tc:
        with (
            tc.tile_pool(name="singles", bufs=1) as singles,
            tc.tile_pool(name="psE", bufs=1, space="PSUM") as psE,
            tc.tile_pool(name="psV", bufs=1, space="PSUM") as psV,
            tc.tile_pool(name="psG", bufs=2, space="PSUM") as psG,
            tc.tile_pool(name="psPT", bufs=1, space="PSUM") as psPT,
        ):
            ident128 = singles.tile([_P, _P], f32, tag="identf")
            make_identity(nc, ident128)
            ident = ident128[0:_NM, 0:_NM]
            ones128 = singles.tile([_P, 1], f32, tag="ones128")
            nc.vector.memset(ones128, 1.0)
            ones_row = singles.tile([1, _P], f32, tag="ones_row")
            nc.vector.memset(ones_row, 1.0)
            onesN = singles.tile([_P, 1], f32, tag="onesN")
            nc.vector.memset(onesN, 1.0 / float(_N) ** 0.5)
            # dummy Exp so the activation-table load runs during the DMA wait
            warm = singles.tile([1, 1], f32, tag="warm")
            nc.scalar.activation(out=warm, in_=ones128[0:1, 0:1], func=Act.Exp)

            # ---- two input DMAs: particle-major tiles + feature-major rows
            mstatall = singles.tile([_P, _NT, _D], f32, tag="mstatall")
            nc.sync.dma_start(
                out=mstatall, in_=p_in[:].rearrange("(c i) d -> i c d", c=_NT)
            )
            msqall = singles.tile([_P, _NT, _D], f32, tag="msqall")
            nc.vector.tensor_mul(msqall, mstatall, mstatall)

            # ---- stats on the PE: two sequential accumulation groups
            # (sum p, then sum p^2), each copied out and transposed to a
            # row so the var chain runs at partition 0
            sv4 = []
            for g, (src, rv) in enumerate(((mstatall, onesN), (msqall, ones128))):
                st4 = psE.tile([_D, 1], f32, tag="early")
                for c in range(_NT):
                    nc.tensor.matmul(
                        st4, lhsT=src[:, c, :], rhs=rv,
                        start=(c == 0), stop=(c == _NT - 1),
                    )
                sv = singles.tile([_D, 1], f32, tag=f"sv4_{g}")
                nc.vector.tensor_copy(sv, st4)
                sv4.append(sv)
            # den = sum(p^2) - sum(p)^2/n = (n-1) var; phcol = 0.5/pilot^2
            den = singles.tile([_D, 1], f32, tag="den")
            nc.vector.tensor_mul(den, sv4[0], sv4[0])
            nc.vector.tensor_sub(den, sv4[1], den)
            var_t = singles.tile([_D, 1], f32, tag="var_t")
            nc.vector.tensor_scalar_mul(var_t, den, 1.0 / (_N - 1))
            nc.sync.dma_start(out=var_out[:], in_=var_t)
            denf = singles.tile([_D, 1], f32, tag="denf")
            nc.vector.tensor_scalar_mul(
                denf, den, 2.0 * _FACT * _FACT / (_N - 1)
            )
            phcol = singles.tile([_D, 1], f32, tag="phcol")
            nc.vector.reciprocal(phcol, denf)

            # QTr = p in feature-major f32r via 8 PE transposes of the
            # tile-major data (no second DMA); Qs = QTr * 2*phcol
            QTr = singles.tile([_D, _N], fr, tag="qtr")
            for c in range(_NT):
                cs = slice(c * _P, (c + 1) * _P)
                ps_q = psG.tile([_D, _P], f32, tag="psg")
                nc.tensor.transpose(ps_q, mstatall[:, c, :], ident128)
                nc.vector.tensor_copy(QTr[:, cs], ps_q)
            Qs = singles.tile([_D, _N], fr, tag="qs")
            nc.vector.tensor_scalar(
                out=Qs, in0=QTr, scalar1=phcol, scalar2=2.0,
                op0=Alu.mult, op1=Alu.mult,
            )

            # 0.5/pilot^2 as a row + broadcast to [128,4] via rank-1 PE
            # outer product (for the r_i reductions)
            ps_pr = psE.tile([1, _D], f32, tag="early")
            nc.tensor.transpose(ps_pr, phcol, ident[0:_D, 0:_D])
            ph_r = singles.tile([1, _D], f32, tag="ph_r")
            nc.vector.tensor_copy(ph_r, ps_pr)
            ps_bc = psE.tile([_P, _D], f32, tag="early")
            nc.tensor.matmul(ps_bc, lhsT=ones_row, rhs=ph_r, start=True, stop=True)
            bc_sb = singles.tile([_P, _D], f32, tag="bc_sb")
            nc.vector.tensor_copy(bc_sb, ps_bc)

            # ---- exp bias nhall[:, c] = -r/2 = -sum_d p^2 * (0.5/pilot^2)
            nhall = singles.tile([_P, _NT], f32, tag="nhall")
            scr = singles.tile([_P, _NT, _D], f32, tag="scr")
            for c in range(_NT):
                nc.vector.tensor_mul(scr[:, c, :], msqall[:, c, :], bc_sb)
                nc.vector.tensor_reduce(
                    out=nhall[:, c : c + 1], in_=scr[:, c, :],
                    axis=Ax.X, op=Alu.add, negate=True,
                )

            # ---- Mp tiles (f32r, PT-stage weights) built in strided copies;
            # MX = Mp . e^{-r/2} per tile
            mtall = singles.tile([_P, _NT, _NM], fr, tag="mtall")
            for c in range(_NT):
                nc.vector.tensor_copy(mtall[:, c, 0:1], ones128)
            nc.vector.tensor_copy(mtall[:, :, 1 : 1 + _D], mstatall)
            nc.vector.tensor_copy(mtall[:, :, 1 + _D : _NM], msqall)
            cneg = singles.tile([_P, _NT], f32, tag="cneg")
            nc.scalar.activation(out=cneg, in_=nhall, func=Act.Exp)
            mxall = singles.tile([_P, _NT, _NM], f32, tag="mxall")
            for c in range(_NT):
                nc.vector.tensor_scalar_mul(
                    mxall[:, c, :], mtall[:, c, :], cneg[:, c : c + 1]
                )

            # ---- main stream: per row tile, two f32r Gram matmuls into a
            # 2-bank PSUM tile, one [128,1024] Exp, then the tile's PT
            # contributions (both j-half accumulation groups chase the exps)
            KT = singles.tile([_P, _NT, _N], fr, tag="kt")
            pspt = psPT.tile([_NM, 2, 512], f32, tag="pspt")
            for ir in range(_NT):
                irs = slice(ir * _P, (ir + 1) * _P)
                psg = psG.tile([_P, 2, 512], f32, tag="psg")
                for jh in range(2):
                    js = slice(jh * 512, (jh + 1) * 512)
                    nc.tensor.matmul(
                        psg[:, jh, :],
                        lhsT=Qs[:, irs],
                        rhs=QTr[:, js],
                        start=True, stop=True,
                    )
                nc.scalar.activation(
                    out=KT[:, ir, :],
                    in_=psg.rearrange("p a b -> p (a b)"),
                    func=Act.Exp,
                    bias=nhall[:, ir : ir + 1],
                )
                for jh in range(2):
                    js = slice(jh * 512, (jh + 1) * 512)
                    nc.tensor.matmul(
                        pspt[:, jh, :],
                        lhsT=mtall[:, ir, :],
                        rhs=KT[:, ir, js],
                        start=(ir == 0), stop=(ir == _NT - 1),
                        skip_group_check=True,
                    )

            # ---- PT out of PSUM, P'' = PT^T per block into one PSUM bank,
            # V = MX^T P''
            pts = singles.tile([_NM, _N], f32, tag="pts")
            nc.vector.tensor_copy(pts[:, 0:512], pspt[:, 0, :])
            nc.vector.tensor_copy(pts[:, 512:1024], pspt[:, 1, :])
            psp2 = psE.tile([_P, _NT, _NM], f32, tag="early")
            for r in range(_NT):
                nc.tensor.transpose(
                    psp2[:, r, :], pts[:, r * _P : (r + 1) * _P], ident
                )
            prall = singles.tile([_P, _NT, _NM], f32, tag="prall")
            nc.vector.tensor_copy(prall, psp2)
            psv = psV.tile([_NM, _NM], f32, tag="psv")
            for r in range(_NT):
                nc.tensor.matmul(
                    psv, lhsT=mxall[:, r, :], rhs=prall[:, r, :],
                    start=(r == 0), stop=(r == _NT - 1),
                )
            Vt = singles.tile([_NM, _NM], f32, tag="vt")
            nc.vector.tensor_copy(Vt, psv)
            nc.sync.dma_start(out=v_out[:], in_=Vt)

    nc.compile()
    return nc


def _get_nc():
    global _NC
    if _NC is None:
        _NC = _build_kernel()
    return _NC


def finalize(V, var):
    """Host-side tail: V [9,9] (raw-p units), var [4] -> bandwidth [4]."""
    V = V.astype(np.float64)
    var = var.astype(np.float64).reshape(_D)
    pilot = _FACT * np.sqrt(var)
    d = np.arange(_D)
    s2 = (
        (V[0, 5 + d] + V[5 + d, 0] - 2.0 * V[1 + d, 1 + d]) / pilot**2 - V[0, 0]
    ) * _INV_SQRT_2PI
    denom = _N * (_N - 1)
    I2 = s2 / pilot**5 / denom
    J1 = _RK / I2
    base = J1 / _N
    return (np.sign(base) * np.abs(base) ** 0.2).astype(np.float32)


def kernel(particles, weights=None, **_unused):
    from concourse.bass_utils import run_bass_kernel_spmd

    particles = np.ascontiguousarray(np.asarray(particles), dtype=np.float32)
    assert particles.shape == (_B, _N, _D), particles.shape

    nc = _get_nc()
    in_maps = [{"p": particles[c]} for c in range(_B)]
    res = run_bass_kernel_spmd(nc, in_maps, list(range(_B)))

    out = np.empty((_B, _D), np.float32)
    for c in range(_B):
        out[c] = finalize(res.results[c]["vout"], res.results[c]["varout"])
    return out



# revision 11
# speedup vs baseline: 1.2967x; 1.2967x over previous
"""Trainium2 Bass kernel for nn_BandwidthPredictorNNHall.

Math: for each batch b (8 of them, one per NeuronCore) with particles
x [n=1024, d=4]:
    pilot_d = 1.0592 * std(x_d, ddof=1) * n^(-1/8)
    q = x / pilot,   K_ij = exp(-0.5 * |q_i - q_j|^2)
    s2_d = sum_ij K_ij ((q_jd - q_id)^2 - 1)
    s3-based bandwidth2 is exactly 0 (odd under i<->j) and treated as 0.
With Mp = [1, p, p^2] (n x 9, RAW particle units), every sum needed for
s2 is an entry of V = Mp^T K Mp; the host applies ~30 scalar flops.

Device pipeline per core (vs the v1 kernel: triangle exp + transpose-free
reduction):
  - One contiguous 16KB input DMA [128, 8, 4]; particle order is
    irrelevant (all reductions are pair-permutation-invariant), so the
    fastest descriptor layout wins.
  - Stats (sum p, sum p^2) as two tiny PE accumulation groups; the var ->
    1/pilot^2 chain needs only a reciprocal (no sqrt => single {Exp,Copy}
    activation table).
  - Feature-major operands: one PE transpose gives T32 [32, 128] (raw,
    partition-sliced per row tile for lhsT); 8 PE transposes + a
    ScalarE/DVE split scaled copy give QTrs = (p * 1/pilot^2)^T [4, 1024]
    (rhs). G_ij = sum_d p_id * (s2_d p_jd) = q_i . q_j.
  - K'' = exp(G - r_i/2) computed ONLY for the upper-triangle blocks
    (row tile I covers columns j >= 128 I): 589k exps instead of 1M.
    K''_ij = K_ij e^{+r_j/2}; the column scale cancels later.
  - Reduction with no PT transposes: for each stored block B(I,J),
        psW[:,J,:] += matmul(lhsT=B, rhs=Mp[I])   (= B^T Mp[I], [128, 9])
    using 0.5*Mp for the diagonal I==J block. After group J closes,
        Vs += matmul(lhsT=W_J, rhs=MX[J]),  MX = Mp . e^{-r_j/2}
    which cancels the e^{+r_j/2} column scale exactly. By block symmetry
    of the true K, V = Vs + Vs^T (computed on the host in f64).
"""

import sys

sys.path.insert(0, "/opt/trn_rl_repo")

import numpy as np

_B, _N, _D = 8, 1024, 4
_P = 128
_NT = _N // _P  # 8 row tiles
_NM = 1 + 2 * _D  # 9 basis columns: [1, p, p^2]
_INV_SQRT_2PI = 1.0 / np.sqrt(2.0 * np.pi)
_RK = 0.282095
_FACT = 1.0592 * float(_N) ** (-1.0 / (4 + _D))

# row tile I covers columns [128*I, 1024); KT column offset per tile
_W = [_N - _P * i for i in range(_NT)]
_OFF = [0] * _NT
for _i in range(1, _NT):
    _OFF[_i] = _OFF[_i - 1] + _W[_i - 1]
_KTW = sum(_W)  # 4608

_NC = None  # compiled Bass module cache


def _build_kernel():
    import concourse.bass as bass  # noqa: F401
    import concourse.tile as tile
    from concourse import bacc, mybir
    from concourse.masks import make_identity

    f32 = mybir.dt.float32
    fr = mybir.dt.float32r
    Act = mybir.ActivationFunctionType
    Alu = mybir.AluOpType
    Ax = mybir.AxisListType

    nc = bacc.Bacc("TRN2", target_bir_lowering=False, debug=False, num_devices=_B)
    p_in = nc.dram_tensor("p", [_N, _D], f32, kind="ExternalInput")
    s_out = nc.dram_tensor("sout", [_NM, _NM], f32, kind="ExternalOutput")
    var_out = nc.dram_tensor("varout", [_D, 1], f32, kind="ExternalOutput")

    with tile.TileContext(nc) as tc:
        with (
            tc.tile_pool(name="singles", bufs=1) as singles,
            tc.tile_pool(name="psBig", bufs=2, space="PSUM") as psBig,
            tc.tile_pool(name="psW", bufs=1, space="PSUM") as psWp,
            tc.tile_pool(name="psV", bufs=1, space="PSUM") as psVp,
            tc.tile_pool(name="psE", bufs=1, space="PSUM") as psE,
        ):
            # ---- input DMA first: contiguous 16KB, particle (8i+r) ->
            # (partition i, slot r); any particle<->(tile,lane) bijection
            # is valid for the pairwise sums.
            mstat = singles.tile([_P, _NT, _D], f32, tag="mstat")
            nc.sync.dma_start(
                out=mstat, in_=p_in[:].rearrange("(i r) d -> i r d", i=_P)
            )

            # ---- constants (Pool/DVE, overlap the DMA wait)
            ident128 = singles.tile([_P, _P], f32, tag="identf")
            make_identity(nc, ident128)
            ones128 = singles.tile([_P, 1], f32, tag="ones128")
            nc.vector.memset(ones128, 1.0)
            halfrow = singles.tile([1, _P], f32, tag="halfrow")
            nc.vector.memset(halfrow, 0.5)
            mtall = singles.tile([_P, _NT, _NM], f32, tag="mtall")
            nc.vector.memset(mtall[:, :, 0:1], 1.0)
            # dummy Exp so the activation-table load runs during the DMA wait
            warm = singles.tile([1, 1], f32, tag="warm")
            nc.scalar.activation(out=warm, in_=ones128[0:1, 0:1], func=Act.Exp)

            # ---- squares
            msq = singles.tile([_P, _NT, _D], f32, tag="msq")
            nc.vector.tensor_mul(msq, mstat, mstat)

            # ---- stats on PE: psS[:,0] = sum p, psS[:,1] = sum p^2
            psSm = psE.tile([_P, 12], f32, tag="psSm")
            psS = psSm[0:_D, 0:2]
            for t in range(_NT):
                nc.tensor.matmul(
                    psS[:, 0:1], lhsT=mstat[:, t, :], rhs=ones128,
                    start=(t == 0), stop=(t == _NT - 1), skip_group_check=True,
                )
            for t in range(_NT):
                nc.tensor.matmul(
                    psS[:, 1:2], lhsT=msq[:, t, :], rhs=ones128,
                    start=(t == 0), stop=(t == _NT - 1), skip_group_check=True,
                )
            # feature-major rhs staging: 8 transposes into one 2-bank tile
            psQ = psBig.tile([_P, _N], f32, tag="psg")
            for c in range(_NT):
                nc.tensor.transpose(
                    psQ[0:_D, c * _P : (c + 1) * _P], mstat[:, c, :], ident128
                )

            # ---- var chain (DVE): s2col = 1/pilot^2 as a [4,1] column
            sums = singles.tile([_D, 2], f32, tag="sums")
            nc.vector.tensor_copy(sums, psS)
            t1 = singles.tile([_D, 1], f32, tag="t1")
            nc.vector.tensor_scalar(
                out=t1, in0=sums[:, 0:1], scalar1=sums[:, 0:1],
                scalar2=-1.0 / _N, op0=Alu.mult, op1=Alu.mult,
            )
            den = singles.tile([_D, 1], f32, tag="den")
            nc.vector.tensor_add(den, t1, sums[:, 1:2])  # (n-1) var
            var_t = singles.tile([_D, 1], f32, tag="var_t")
            nc.vector.tensor_scalar_mul(var_t, den, 1.0 / (_N - 1))
            nc.sync.dma_start(out=var_out[:], in_=var_t)
            denf = singles.tile([_D, 1], f32, tag="denf")
            nc.vector.tensor_scalar_mul(denf, den, _FACT * _FACT / (_N - 1))
            s2col = singles.tile([_D, 1], f32, tag="s2col")
            nc.vector.reciprocal(s2col, denf)  # 1/pilot^2

            # ---- raw rhs QTrr = p^T [4, 1024]: split the PSUM->SBUF copy
            # across ScalarE and DVE (no stats dependency -> early);
            # scaled lhsT slices QTls[:, I, :] = (s2 * p)^T per row tile
            # built on the otherwise-idle Pool engine.
            QTrr = singles.tile([_D, _N], fr, tag="qtrr")
            nc.scalar.copy(QTrr[:, 0:512], psQ[0:_D, 0:512])
            nc.vector.tensor_copy(QTrr[:, 512:_N], psQ[0:_D, 512:_N])
            QTls = singles.tile([_D, _NT, _P], fr, tag="qtls")

            # two early scaled lhsT slices on ScalarE (after its raw half
            # copy), the rest on DVE spread through the stream
            for c in range(2):
                nc.scalar.mul(
                    QTls[:, c, :], psQ[0:_D, c * _P : (c + 1) * _P], s2col
                )

            # ---- nhall = -r/2 per particle: rank-1 PE broadcasts; DVE
            # multiplies msq by the PSUM broadcast row directly
            psrow = psSm[0:1, 4:8]
            nc.tensor.matmul(
                psrow, lhsT=s2col, rhs=ident128[0:_D, 0:_D],
                start=True, stop=True, skip_group_check=True,
            )
            s2row = singles.tile([1, _D], f32, tag="s2row")
            nc.vector.tensor_copy(s2row, psrow)
            psbc = psSm[:, 8:12]
            nc.tensor.matmul(
                psbc, lhsT=halfrow, rhs=s2row, start=True, stop=True,
                skip_group_check=True,
            )  # 0.5 * s2_d broadcast to all partitions
            scr = singles.tile([_P, _NT, _D], f32, tag="scr")
            nc.vector.tensor_mul(
                scr, msq, psbc.unsqueeze(1).broadcast_to((_P, _NT, _D))
            )
            nhall = singles.tile([_P, _NT], f32, tag="nhall")
            nc.vector.tensor_reduce(
                out=nhall, in_=scr, axis=Ax.X, op=Alu.add, negate=True
            )
            for c in range(2, _NT):
                nc.vector.tensor_scalar_mul(
                    QTls[:, c, :], psQ[0:_D, c * _P : (c + 1) * _P], s2col
                )

            # ---- Mp tiles (fr) on the idle Pool engine: [1 | p | p^2];
            # mthalf for the diagonal blocks; cneg = e^{-r/2} cancels the
            # stored K'' column scale inside the wsb copies
            nc.gpsimd.tensor_copy(mtall[:, :, 1 : 1 + _D], mstat)
            nc.gpsimd.tensor_copy(mtall[:, :, 1 + _D : _NM], msq)
            mthalf = singles.tile([_P, _NT, _NM], f32, tag="mthalf")
            nc.gpsimd.tensor_scalar_mul(mthalf, mtall, 0.5)
            cneg = singles.tile([_P, _NT], f32, tag="cneg")
            nc.scalar.activation(out=cneg, in_=nhall, func=Act.Exp)

            # ---- main stream: per row tile I (ascending), Gram chunks for
            # columns [128I, 1024) -> one exp -> per-block W matmuls; the
            # W group J closes at I==J, then its Vs contribution fires.
            KT = singles.tile([_P, _KTW], f32, tag="kt")
            psW = psWp.tile([_P, _NT, _NM], f32, tag="psw")
            psVs = psVp.tile([_NM, _NM], f32, tag="psv")
            wsb = singles.tile([_P, _NT, _NM], f32, tag="wsb")
            psg_t = [None] * _NT

            def g_chunks(i):
                cs = _P * i
                if cs < 512:
                    return [(cs, 512), (512, _N)]
                return [(cs, _N)]

            def emit_g(i):
                psg = psBig.tile([_P, _N], f32, tag="psg")
                psg_t[i] = psg
                for a, b in g_chunks(i):
                    nc.tensor.matmul(
                        psg[:, a:b],
                        lhsT=QTls[:, i, :],
                        rhs=QTrr[:, a:b],
                        start=True, stop=True,
                    )

            def emit_w(j):
                # after act J: group J = blocks (I, J) for I <= J, emitted
                # contiguously (interleaved PSUM groups corrupt the bank)
                for i in range(j + 1):
                    rhs = mthalf[:, i, :] if i == j else mtall[:, i, :]
                    nc.tensor.matmul(
                        psW[:, j, :],
                        lhsT=KT[:, _OFF[i] + _P * (j - i) : _OFF[i] + _P * (j - i + 1)],
                        rhs=rhs,
                        start=(i == 0), stop=(i == j), skip_group_check=True,
                    )

            def emit_vs(j):
                # e^{-r_j/2} per partition cancels the K'' column scale
                nc.vector.tensor_scalar_mul(
                    wsb[:, j, :], psW[:, j, :], cneg[:, j : j + 1]
                )
                nc.tensor.matmul(
                    psVs, lhsT=wsb[:, j, :], rhs=mtall[:, j, :],
                    start=(j == 0), stop=(j == _NT - 1),
                )

            emit_g(0)
            emit_g(1)
            for i in range(_NT):
                cs = _P * i
                nc.scalar.activation(
                    out=KT[:, _OFF[i] : _OFF[i] + _W[i]],
                    in_=psg_t[i][:, cs:_N],
                    func=Act.Exp,
                    bias=nhall[:, i : i + 1],
                )
                if i + 2 < _NT:
                    emit_g(i + 2)
                emit_w(i)
                emit_vs(i)

            Vt = singles.tile([_NM, _NM], f32, tag="vt")
            nc.vector.tensor_copy(Vt, psVs)
            nc.sync.dma_start(out=s_out[:], in_=Vt)

    nc.compile()
    return nc


def _get_nc():
    global _NC
    if _NC is None:
        _NC = _build_kernel()
    return _NC


def finalize(S, var):
    """Host-side tail: S [9,9] (V = S + S^T, raw-p units), var [4] ->
    bandwidth [4]."""
    S = S.astype(np.float64)
    V = S + S.T
    var = var.astype(np.float64).reshape(_D)
    pilot = _FACT * np.sqrt(var)
    d = np.arange(_D)
    s2 = (
        (V[0, 5 + d] + V[5 + d, 0] - 2.0 * V[1 + d, 1 + d]) / pilot**2 - V[0, 0]
    ) * _INV_SQRT_2PI
    denom = _N * (_N - 1)
    I2 = s2 / pilot**5 / denom
    J1 = _RK / I2
    base = J1 / _N
    return (np.sign(base) * np.abs(base) ** 0.2).astype(np.float32)


def kernel(particles, weights=None, **_unused):
    from concourse.bass_utils import run_bass_kernel_spmd

    particles = np.ascontiguousarray(np.asarray(particles), dtype=np.float32)
    assert particles.shape == (_B, _N, _D), particles.shape

    nc = _get_nc()
    in_maps = [{"p": particles[c]} for c in range(_B)]
    res = run_bass_kernel_spmd(nc, in_maps, list(range(_B)))

    out = np.empty((_B, _D), np.float32)
    for c in range(_B):
        out[c] = finalize(res.results[c]["sout"], res.results[c]["varout"])
    return out


# revision 12
# speedup vs baseline: 1.3899x; 1.0719x over previous
"""Trainium2 Bass kernel for nn_BandwidthPredictorNNHall.

Math: for each batch b (8 of them, one per NeuronCore) with particles
x [n=1024, d=4]:
    pilot_d = 1.0592 * std(x_d, ddof=1) * n^(-1/8)
    q = x / pilot,   K_ij = exp(-0.5 * |q_i - q_j|^2)
    s2_d = sum_ij K_ij ((q_jd - q_id)^2 - 1)
    s3-based bandwidth2 is exactly 0 (odd under i<->j) and treated as 0.
With Mp = [1, p, p^2] (n x 9, RAW particle units), every sum needed for
s2 is an entry of V = Mp^T K Mp; the host applies ~30 scalar flops.

Device pipeline per core (vs the v1 kernel: triangle exp + transpose-free
reduction):
  - One contiguous 16KB input DMA [128, 8, 4]; particle order is
    irrelevant (all reductions are pair-permutation-invariant), so the
    fastest descriptor layout wins.
  - Stats (sum p, sum p^2) as two tiny PE accumulation groups; the var ->
    1/pilot^2 chain needs only a reciprocal (no sqrt => single {Exp,Copy}
    activation table).
  - Feature-major operands: one PE transpose gives T32 [32, 128] (raw,
    partition-sliced per row tile for lhsT); 8 PE transposes + a
    ScalarE/DVE split scaled copy give QTrs = (p * 1/pilot^2)^T [4, 1024]
    (rhs). G_ij = sum_d p_id * (s2_d p_jd) = q_i . q_j.
  - K'' = exp(G - r_i/2) computed ONLY for the upper-triangle blocks
    (row tile I covers columns j >= 128 I): 589k exps instead of 1M.
    K''_ij = K_ij e^{+r_j/2}; the column scale cancels later.
  - Reduction with no PT transposes: for each stored block B(I,J),
        psW[:,J,:] += matmul(lhsT=B, rhs=Mp[I])   (= B^T Mp[I], [128, 9])
    using 0.5*Mp for the diagonal I==J block. After group J closes,
        Vs += matmul(lhsT=W_J, rhs=MX[J]),  MX = Mp . e^{-r_j/2}
    which cancels the e^{+r_j/2} column scale exactly. By block symmetry
    of the true K, V = Vs + Vs^T (computed on the host in f64).
"""

import sys

sys.path.insert(0, "/opt/trn_rl_repo")

import numpy as np

_B, _N, _D = 8, 1024, 4
_P = 128
_NT = _N // _P  # 8 row tiles
_NM = 1 + 2 * _D  # 9 basis columns: [1, p, p^2]
_INV_SQRT_2PI = 1.0 / np.sqrt(2.0 * np.pi)
_RK = 0.282095
_FACT = 1.0592 * float(_N) ** (-1.0 / (4 + _D))

# row tile I covers columns [128*I, 1024); KT column offset per tile
_W = [_N - _P * i for i in range(_NT)]
_OFF = [0] * _NT
for _i in range(1, _NT):
    _OFF[_i] = _OFF[_i - 1] + _W[_i - 1]
_KTW = sum(_W)  # 4608

_NC = None  # compiled Bass module cache


def _build_kernel():
    import concourse.bass as bass  # noqa: F401
    import concourse.tile as tile
    from concourse import bacc, mybir
    from concourse.masks import make_identity

    f32 = mybir.dt.float32
    fr = mybir.dt.float32r
    Act = mybir.ActivationFunctionType
    Alu = mybir.AluOpType
    Ax = mybir.AxisListType

    nc = bacc.Bacc("TRN2", target_bir_lowering=False, debug=False, num_devices=_B)
    p_in = nc.dram_tensor("p", [_N, _D], f32, kind="ExternalInput")
    s_out = nc.dram_tensor("sout", [_NM, _NM], f32, kind="ExternalOutput")
    var_out = nc.dram_tensor("varout", [_D, 1], f32, kind="ExternalOutput")

    with tile.TileContext(nc) as tc:
        with (
            tc.tile_pool(name="singles", bufs=1) as singles,
            tc.tile_pool(name="psBig", bufs=2, space="PSUM") as psBig,
            tc.tile_pool(name="psW", bufs=1, space="PSUM") as psWp,
            tc.tile_pool(name="psV", bufs=1, space="PSUM") as psVp,
            tc.tile_pool(name="psE", bufs=1, space="PSUM") as psE,
        ):
            # ---- input DMA first: contiguous 16KB, particle (8i+r) ->
            # (partition i, slot r); any particle<->(tile,lane) bijection
            # is valid for the pairwise sums.
            mstat = singles.tile([_P, _NT, _D], f32, tag="mstat")
            nc.sync.dma_start(
                out=mstat, in_=p_in[:].rearrange("(i r) d -> i r d", i=_P)
            )

            # ---- constants (Pool/DVE, overlap the DMA wait)
            ident128 = singles.tile([_P, _P], f32, tag="identf")
            make_identity(nc, ident128)
            ones128 = singles.tile([_P, 1], f32, tag="ones128")
            nc.vector.memset(ones128, 1.0)
            half4 = singles.tile([_D, _P], f32, tag="half4")
            nc.vector.memset(half4, 0.5)
            mtall = singles.tile([_P, _NT, _NM], f32, tag="mtall")
            nc.vector.memset(mtall[:, :, 0:1], 1.0)
            # dummy Exp so the activation-table load runs during the DMA wait
            warm = singles.tile([1, 1], f32, tag="warm")
            nc.scalar.activation(out=warm, in_=ones128[0:1, 0:1], func=Act.Exp)

            # ---- squares
            msq = singles.tile([_P, _NT, _D], f32, tag="msq")
            nc.vector.tensor_mul(msq, mstat, mstat)

            # ---- stats on PE: psS[:,0] = sum p, psS[:,1] = sum p^2
            psSm = psE.tile([_P, 12], f32, tag="psSm")
            psS = psSm[0:_D, 0:2]
            for t in range(_NT):
                nc.tensor.matmul(
                    psS[:, 0:1], lhsT=mstat[:, t, :], rhs=ones128,
                    start=(t == 0), stop=(t == _NT - 1), skip_group_check=True,
                )
            for t in range(_NT):
                nc.tensor.matmul(
                    psS[:, 1:2], lhsT=msq[:, t, :], rhs=ones128,
                    start=(t == 0), stop=(t == _NT - 1), skip_group_check=True,
                )
            # feature-major rhs staging: 8 transposes into one 2-bank tile
            psQ = psBig.tile([_P, _N], f32, tag="psg")
            for c in range(_NT):
                nc.tensor.transpose(
                    psQ[0:_D, c * _P : (c + 1) * _P], mstat[:, c, :], ident128
                )

            # ---- var chain (DVE): s2col = 1/pilot^2 as a [4,1] column
            sums = singles.tile([_D, 2], f32, tag="sums")
            nc.vector.tensor_copy(sums, psS)
            t1 = singles.tile([_D, 1], f32, tag="t1")
            nc.vector.tensor_scalar(
                out=t1, in0=sums[:, 0:1], scalar1=sums[:, 0:1],
                scalar2=-1.0 / _N, op0=Alu.mult, op1=Alu.mult,
            )
            den = singles.tile([_D, 1], f32, tag="den")
            nc.vector.tensor_add(den, t1, sums[:, 1:2])  # (n-1) var
            var_t = singles.tile([_D, 1], f32, tag="var_t")
            nc.vector.tensor_scalar_mul(var_t, den, 1.0 / (_N - 1))
            nc.sync.dma_start(out=var_out[:], in_=var_t)
            denf = singles.tile([_D, 1], f32, tag="denf")
            nc.vector.tensor_scalar_mul(denf, den, _FACT * _FACT / (_N - 1))
            s2col = singles.tile([_D, 1], f32, tag="s2col")
            nc.vector.reciprocal(s2col, denf)  # 1/pilot^2

            # ---- raw rhs QTrr = p^T [4, 1024]: split the PSUM->SBUF copy
            # across ScalarE and DVE (no stats dependency -> early)
            QTrr = singles.tile([_D, _N], fr, tag="qtrr")
            nc.scalar.copy(QTrr[:, 0:512], psQ[0:_D, 0:512])
            nc.vector.tensor_copy(QTrr[:, 512:_N], psQ[0:_D, 512:_N])

            # ---- nhall = -r/2 per particle: diag(s2) via one DVE op, one
            # rank-1 PE broadcast, then multiply/reduce; no row-form hops
            diag4 = singles.tile([_D, _D], f32, tag="diag4")
            nc.vector.tensor_scalar_mul(diag4, ident128[0:_D, 0:_D], s2col)
            psbc = psSm[:, 8:12]
            nc.tensor.matmul(
                psbc, lhsT=half4, rhs=diag4, start=True, stop=True,
                skip_group_check=True,
            )  # 0.5 * s2_d broadcast to all partitions
            scr = singles.tile([_P, _NT, _D], f32, tag="scr")
            nc.vector.tensor_mul(
                scr, msq, psbc.unsqueeze(1).broadcast_to((_P, _NT, _D))
            )
            nhall = singles.tile([_P, _NT], f32, tag="nhall")
            nc.vector.tensor_reduce(
                out=nhall, in_=scr, axis=Ax.X, op=Alu.add, negate=True
            )
            cneg = singles.tile([_P, _NT], f32, tag="cneg")
            nc.scalar.activation(out=cneg, in_=nhall, func=Act.Exp)

            # ---- scaled lhsT slices QTls[:, c, :] = (s2 * p)^T per row
            # tile, from SBUF QTrr on the otherwise-idle Pool engine; Mp
            # tiles [1 | p | p^2] and mthalf (diagonal blocks) too
            QTls = singles.tile([_D, _NT, _P], fr, tag="qtls")
            for c in range(2):
                nc.gpsimd.tensor_scalar_mul(
                    QTls[:, c, :], QTrr[:, c * _P : (c + 1) * _P], s2col
                )
            nc.gpsimd.tensor_copy(mtall[:, :, 1 : 1 + _D], mstat)
            nc.gpsimd.tensor_copy(mtall[:, :, 1 + _D : _NM], msq)
            mthalf = singles.tile([_P, _NT, _NM], f32, tag="mthalf")
            nc.gpsimd.tensor_scalar_mul(mthalf, mtall, 0.5)
            for c in range(2, _NT):
                nc.gpsimd.tensor_scalar_mul(
                    QTls[:, c, :], QTrr[:, c * _P : (c + 1) * _P], s2col
                )

            # ---- main stream: per row tile I (ascending), Gram chunks for
            # columns [128I, 1024) -> one exp -> per-block W matmuls; the
            # W group J closes at I==J, then its Vs contribution fires.
            KT = singles.tile([_P, _KTW], f32, tag="kt")
            psW = psWp.tile([_P, _NT, _NM], f32, tag="psw")
            psVs = psVp.tile([_NM, _NM], f32, tag="psv")
            wsb = singles.tile([_P, _NT, _NM], f32, tag="wsb")
            psg_t = [None] * _NT

            def g_chunks(i):
                cs = _P * i
                if cs < 512:
                    return [(cs, 512), (512, _N)]
                return [(cs, _N)]

            def emit_g(i):
                psg = psBig.tile([_P, _N], f32, tag="psg")
                psg_t[i] = psg
                for a, b in g_chunks(i):
                    nc.tensor.matmul(
                        psg[:, a:b],
                        lhsT=QTls[:, i, :],
                        rhs=QTrr[:, a:b],
                        start=True, stop=True,
                    )

            def emit_w(j, i_lo=0, i_hi=None):
                # group J = blocks (I, J) for I <= J; groups must not
                # interleave within the PSUM bank (corruption), but a
                # group's own matmuls may be split across emission points
                if i_hi is None:
                    i_hi = j + 1
                for i in range(i_lo, i_hi):
                    rhs = mthalf[:, i, :] if i == j else mtall[:, i, :]
                    nc.tensor.matmul(
                        psW[:, j, :],
                        lhsT=KT[:, _OFF[i] + _P * (j - i) : _OFF[i] + _P * (j - i + 1)],
                        rhs=rhs,
                        start=(i == 0), stop=(i == j), skip_group_check=True,
                    )

            def emit_vs(j):
                # e^{-r_j/2} per partition cancels the K'' column scale
                nc.vector.tensor_scalar_mul(
                    wsb[:, j, :], psW[:, j, :], cneg[:, j : j + 1]
                )
                nc.tensor.matmul(
                    psVs, lhsT=wsb[:, j, :], rhs=mtall[:, j, :],
                    start=(j == 0), stop=(j == _NT - 1),
                )

            emit_g(0)
            emit_g(1)
            for i in range(_NT):
                cs = _P * i
                nc.scalar.activation(
                    out=KT[:, _OFF[i] : _OFF[i] + _W[i]],
                    in_=psg_t[i][:, cs:_N],
                    func=Act.Exp,
                    bias=nhall[:, i : i + 1],
                )
                if i + 2 < _NT:
                    emit_g(i + 2)
                if i < _NT - 1:
                    emit_w(i)
                    if i == _NT - 2:
                        # last group's I<=6 blocks are all ready now
                        emit_w(_NT - 1, 0, _NT - 1)
                    emit_vs(i)
                else:
                    emit_w(i, _NT - 1, _NT)
                    emit_vs(i)

            Vt = singles.tile([_NM, _NM], f32, tag="vt")
            nc.vector.tensor_copy(Vt, psVs)
            nc.sync.dma_start(out=s_out[:], in_=Vt)

    nc.compile()
    return nc


def _get_nc():
    global _NC
    if _NC is None:
        _NC = _build_kernel()
    return _NC


def finalize(S, var):
    """Host-side tail: S [9,9] (V = S + S^T, raw-p units), var [4] ->
    bandwidth [4]."""
    S = S.astype(np.float64)
    V = S + S.T
    var = var.astype(np.float64).reshape(_D)
    pilot = _FACT * np.sqrt(var)
    d = np.arange(_D)
    s2 = (
        (V[0, 5 + d] + V[5 + d, 0] - 2.0 * V[1 + d, 1 + d]) / pilot**2 - V[0, 0]
    ) * _INV_SQRT_2PI
    denom = _N * (_N - 1)
    I2 = s2 / pilot**5 / denom
    J1 = _RK / I2
    base = J1 / _N
    return (np.sign(base) * np.abs(base) ** 0.2).astype(np.float32)


def kernel(particles, weights=None, **_unused):
    from concourse.bass_utils import run_bass_kernel_spmd

    particles = np.ascontiguousarray(np.asarray(particles), dtype=np.float32)
    assert particles.shape == (_B, _N, _D), particles.shape

    nc = _get_nc()
    in_maps = [{"p": particles[c]} for c in range(_B)]
    res = run_bass_kernel_spmd(nc, in_maps, list(range(_B)))

    out = np.empty((_B, _D), np.float32)
    for c in range(_B):
        out[c] = finalize(res.results[c]["sout"], res.results[c]["varout"])
    return out


# revision 18
# speedup vs baseline: 1.4040x; 1.0101x over previous
"""Trainium2 Bass kernel for nn_BandwidthPredictorNNHall.

Math: for each batch b (8 of them, one per NeuronCore) with particles
x [n=1024, d=4]:
    pilot_d = 1.0592 * std(x_d, ddof=1) * n^(-1/8)
    q = x / pilot,   K_ij = exp(-0.5 * |q_i - q_j|^2)
    s2_d = sum_ij K_ij ((q_jd - q_id)^2 - 1)
    s3-based bandwidth2 is exactly 0 (odd under i<->j) and treated as 0.
With Mp = [1, p, p^2] (n x 9, RAW particle units), every sum needed for
s2 is an entry of V = Mp^T K Mp; the host applies ~30 scalar flops.

Device pipeline per core (vs the v1 kernel: triangle exp + transpose-free
reduction):
  - One contiguous 16KB input DMA [128, 8, 4]; particle order is
    irrelevant (all reductions are pair-permutation-invariant), so the
    fastest descriptor layout wins.
  - Stats (sum p, sum p^2) as two tiny PE accumulation groups; the var ->
    1/pilot^2 chain needs only a reciprocal (no sqrt => single {Exp,Copy}
    activation table).
  - Feature-major operands: one PE transpose gives T32 [32, 128] (raw,
    partition-sliced per row tile for lhsT); 8 PE transposes + a
    ScalarE/DVE split scaled copy give QTrs = (p * 1/pilot^2)^T [4, 1024]
    (rhs). G_ij = sum_d p_id * (s2_d p_jd) = q_i . q_j.
  - K'' = exp(G - r_i/2) computed ONLY for the upper-triangle blocks
    (row tile I covers columns j >= 128 I): 589k exps instead of 1M.
    K''_ij = K_ij e^{+r_j/2}; the column scale cancels later.
  - Reduction with no PT transposes: for each stored block B(I,J),
        psW[:,J,:] += matmul(lhsT=B, rhs=Mp[I])   (= B^T Mp[I], [128, 9])
    using 0.5*Mp for the diagonal I==J block. After group J closes,
        Vs += matmul(lhsT=W_J, rhs=MX[J]),  MX = Mp . e^{-r_j/2}
    which cancels the e^{+r_j/2} column scale exactly. By block symmetry
    of the true K, V = Vs + Vs^T (computed on the host in f64).
"""

import sys

sys.path.insert(0, "/opt/trn_rl_repo")

import numpy as np

_B, _N, _D = 8, 1024, 4
_P = 128
_NT = _N // _P  # 8 row tiles
_NM = 1 + 2 * _D  # 9 basis columns: [1, p, p^2]
_INV_SQRT_2PI = 1.0 / np.sqrt(2.0 * np.pi)
_RK = 0.282095
_FACT = 1.0592 * float(_N) ** (-1.0 / (4 + _D))

# row tile I covers columns [128*I, 1024); KT column offset per tile
_W = [_N - _P * i for i in range(_NT)]
_OFF = [0] * _NT
for _i in range(1, _NT):
    _OFF[_i] = _OFF[_i - 1] + _W[_i - 1]
_KTW = sum(_W)  # 4608

_NC = None  # compiled Bass module cache


def _build_kernel():
    import concourse.bass as bass  # noqa: F401
    import concourse.tile as tile
    from concourse import bacc, mybir
    from concourse.masks import make_identity

    f32 = mybir.dt.float32
    fr = mybir.dt.float32r
    Act = mybir.ActivationFunctionType
    Alu = mybir.AluOpType
    Ax = mybir.AxisListType

    nc = bacc.Bacc("TRN2", target_bir_lowering=False, debug=False, num_devices=_B)
    p_in = nc.dram_tensor("p", [_N, _D], f32, kind="ExternalInput")
    s_out = nc.dram_tensor("sout", [_NM, _NM], f32, kind="ExternalOutput")
    var_out = nc.dram_tensor("varout", [_D, 1], f32, kind="ExternalOutput")

    with tile.TileContext(nc) as tc:
        with (
            tc.tile_pool(name="singles", bufs=1) as singles,
            tc.tile_pool(name="psBig", bufs=2, space="PSUM") as psBig,
            tc.tile_pool(name="psW", bufs=2, space="PSUM") as psWp,
            tc.tile_pool(name="psV", bufs=1, space="PSUM") as psVp,
            tc.tile_pool(name="psLate", bufs=1, space="PSUM") as psLate,
        ):
            # ---- input DMA first: contiguous 16KB, particle (8i+r) ->
            # (partition i, slot r); any particle<->(tile,lane) bijection
            # is valid for the pairwise sums.
            mstat = singles.tile([_P, _NT, _D], f32, tag="mstat")
            nc.sync.dma_start(
                out=mstat, in_=p_in[:].rearrange("(i r) d -> i r d", i=_P)
            )

            # ---- constants (Pool/DVE, overlap the DMA wait)
            ident128 = singles.tile([_P, _P], f32, tag="identf")
            make_identity(nc, ident128)
            ones128 = singles.tile([_P, 1], f32, tag="ones128")
            nc.vector.memset(ones128, 1.0)
            half4 = singles.tile([_D, _P], f32, tag="half4")
            nc.vector.memset(half4, 0.5)
            mtall = singles.tile([_P, _NT, _NM], f32, tag="mtall")
            nc.vector.memset(mtall[:, :, 0:1], 1.0)
            # dummy Exp so the activation-table load runs during the DMA wait
            warm = singles.tile([1, 1], f32, tag="warm")
            nc.scalar.activation(out=warm, in_=ones128[0:1, 0:1], func=Act.Exp)

            # ---- squares
            msq = singles.tile([_P, _NT, _D], f32, tag="msq")
            nc.vector.tensor_mul(msq, mstat, mstat)

            # ---- stats on PE: psS[:,0] = sum p, psS[:,1] = sum p^2
            # (regions of the psV bank; all groups in this bank run
            # sequentially: psS x2, psbc, then Vs0..Vs7)
            psVm = psVp.tile([_P, 16], f32, tag="psvm")
            psS = psVm[0:_D, 9:11]
            for t in range(_NT):
                nc.tensor.matmul(
                    psS[:, 0:1], lhsT=mstat[:, t, :], rhs=ones128,
                    start=(t == 0), stop=(t == _NT - 1), skip_group_check=True,
                )
            for t in range(_NT):
                nc.tensor.matmul(
                    psS[:, 1:2], lhsT=msq[:, t, :], rhs=ones128,
                    start=(t == 0), stop=(t == _NT - 1), skip_group_check=True,
                )
            # feature-major rhs staging: 8 transposes into one 2-bank tile
            psQ = psBig.tile([_P, _N], f32, tag="psg")
            for c in range(_NT):
                nc.tensor.transpose(
                    psQ[0:_D, c * _P : (c + 1) * _P], mstat[:, c, :], ident128
                )

            # ---- var chain (DVE): s2col = 1/pilot^2 as a [4,1] column
            sums = singles.tile([_D, 2], f32, tag="sums")
            nc.vector.tensor_copy(sums, psS)
            t1 = singles.tile([_D, 1], f32, tag="t1")
            nc.vector.tensor_scalar(
                out=t1, in0=sums[:, 0:1], scalar1=sums[:, 0:1],
                scalar2=-1.0 / _N, op0=Alu.mult, op1=Alu.mult,
            )
            den = singles.tile([_D, 1], f32, tag="den")
            nc.vector.tensor_add(den, t1, sums[:, 1:2])  # (n-1) var
            var_t = singles.tile([_D, 1], f32, tag="var_t")
            nc.vector.tensor_scalar_mul(var_t, den, 1.0 / (_N - 1))
            nc.sync.dma_start(out=var_out[:], in_=var_t)
            denf = singles.tile([_D, 1], f32, tag="denf")
            nc.vector.tensor_scalar_mul(denf, den, _FACT * _FACT / (_N - 1))
            s2col = singles.tile([_D, 1], f32, tag="s2col")
            nc.vector.reciprocal(s2col, denf)  # 1/pilot^2

            # ---- raw rhs QTrr = p^T [4, 1024]: split the PSUM->SBUF copy
            # across ScalarE and DVE (no stats dependency -> early)
            QTrr = singles.tile([_D, _N], fr, tag="qtrr")
            nc.scalar.copy(QTrr[:, 0:512], psQ[0:_D, 0:512])
            nc.vector.tensor_copy(QTrr[:, 512:_N], psQ[0:_D, 512:_N])

            # ---- nhall = -r/2 per particle: diag(s2) via one DVE op, one
            # rank-1 PE broadcast, then multiply/reduce; no row-form hops
            diag4 = singles.tile([_D, _D], f32, tag="diag4")
            nc.vector.tensor_scalar_mul(diag4, ident128[0:_D, 0:_D], s2col)
            psbc = psVm[:, 12:16]
            nc.tensor.matmul(
                psbc, lhsT=half4, rhs=diag4, start=True, stop=True,
                skip_group_check=True,
            )  # 0.5 * s2_d broadcast to all partitions
            scr = singles.tile([_P, _NT, _D], f32, tag="scr")
            nc.vector.tensor_mul(
                scr, msq, psbc.unsqueeze(1).broadcast_to((_P, _NT, _D))
            )
            nhall = singles.tile([_P, _NT], f32, tag="nhall")
            nc.vector.tensor_reduce(
                out=nhall, in_=scr, axis=Ax.X, op=Alu.add, negate=True
            )
            cneg = singles.tile([_P, _NT], f32, tag="cneg")
            nc.scalar.activation(out=cneg, in_=nhall, func=Act.Exp)

            # ---- scaled lhsT slices QTls[:, c, :] = (s2 * p)^T per row
            # tile, from SBUF QTrr on the otherwise-idle Pool engine; Mp
            # tiles [1 | p | p^2] and mthalf (diagonal blocks) too
            QTls = singles.tile([_D, _NT, _P], fr, tag="qtls")
            for c in range(2):
                nc.gpsimd.tensor_scalar_mul(
                    QTls[:, c, :], QTrr[:, c * _P : (c + 1) * _P], s2col
                )
            nc.gpsimd.tensor_copy(mtall[:, :, 1 : 1 + _D], mstat)
            nc.gpsimd.tensor_copy(mtall[:, :, 1 + _D : _NM], msq)
            mthalf = singles.tile([_P, _NT, _NM], f32, tag="mthalf")
            nc.gpsimd.tensor_scalar_mul(mthalf, mtall, 0.5)
            for c in range(2, _NT):
                nc.gpsimd.tensor_scalar_mul(
                    QTls[:, c, :], QTrr[:, c * _P : (c + 1) * _P], s2col
                )

            # ---- main stream: per row tile I (ascending), Gram chunks for
            # columns [128I, 1024) -> one exp -> per-block W matmuls; the
            # W group J closes at I==J, then its Vs contribution fires.
            KT = singles.tile([_P, _KTW], f32, tag="kt")
            psW_t = [
                psWp.tile([_P, _NM], f32, name=f"psw{j}", tag="psw")
                for j in range(_NT)
            ]
            psVs = psVm[0:_NM, 0:_NM]
            wsb = singles.tile([_P, _NT, _NM], f32, tag="wsb")
            psg_t = [None] * _NT
            psg_base = [0, 0, 0, 0, 0, 640, 768, 896]

            def g_chunks(i):
                cs = _P * i
                if cs < 512:
                    return [(cs, 512), (512, _N)]
                return [(cs, _N)]

            def emit_g(i):
                if i < 5:
                    psg = psBig.tile([_P, _N], f32, tag="psg")
                else:
                    psg = psLate.tile([_P, _N - psg_base[i]], f32, tag="psl")
                psg_t[i] = psg
                for a, b in g_chunks(i):
                    nc.tensor.matmul(
                        psg[:, a - psg_base[i] : b - psg_base[i]],
                        lhsT=QTls[:, i, :],
                        rhs=QTrr[:, a:b],
                        start=True, stop=True,
                    )

            def emit_w(j, i_lo=0, i_hi=None):
                # group J = blocks (I, J) for I <= J; groups must not
                # interleave within the PSUM bank (corruption), but a
                # group's own matmuls may be split across emission points
                if i_hi is None:
                    i_hi = j + 1
                for i in range(i_lo, i_hi):
                    rhs = mthalf[:, i, :] if i == j else mtall[:, i, :]
                    nc.tensor.matmul(
                        psW_t[j],
                        lhsT=KT[:, _OFF[i] + _P * (j - i) : _OFF[i] + _P * (j - i + 1)],
                        rhs=rhs,
                        start=(i == 0), stop=(i == j), skip_group_check=True,
                    )

            def emit_vs(j):
                # e^{-r_j/2} per partition cancels the K'' column scale
                nc.vector.tensor_scalar_mul(
                    wsb[:, j, :], psW_t[j], cneg[:, j : j + 1]
                )
                nc.tensor.matmul(
                    psVs, lhsT=wsb[:, j, :], rhs=mtall[:, j, :],
                    start=(j == 0), stop=(j == _NT - 1),
                )

            emit_g(0)
            emit_g(1)
            for i in range(_NT):
                cs = _P * i
                nc.scalar.activation(
                    out=KT[:, _OFF[i] : _OFF[i] + _W[i]],
                    in_=psg_t[i][:, cs - psg_base[i] : _N - psg_base[i]],
                    func=Act.Exp,
                    bias=nhall[:, i : i + 1],
                )
                if i + 2 < _NT:
                    emit_g(i + 2)
                if i < _NT - 1:
                    emit_w(i)
                    if i == _NT - 2:
                        # last group's I<=6 blocks are all ready now
                        emit_w(_NT - 1, 0, _NT - 1)
                    emit_vs(i)
                else:
                    emit_w(i, _NT - 1, _NT)
                    emit_vs(i)

            Vt = singles.tile([_NM, _NM], f32, tag="vt")
            nc.vector.tensor_copy(Vt, psVs)

            nc.sync.dma_start(out=s_out[:], in_=Vt)

    nc.compile()
    return nc


def _get_nc():
    global _NC
    if _NC is None:
        _NC = _build_kernel()
    return _NC


def finalize(S, var):
    """Host-side tail: S [9,9] (V = S + S^T, raw-p units), var [4] ->
    bandwidth [4]."""
    S = S.astype(np.float64)
    V = S + S.T
    var = var.astype(np.float64).reshape(_D)
    pilot = _FACT * np.sqrt(var)
    d = np.arange(_D)
    s2 = (
        (V[0, 5 + d] + V[5 + d, 0] - 2.0 * V[1 + d, 1 + d]) / pilot**2 - V[0, 0]
    ) * _INV_SQRT_2PI
    denom = _N * (_N - 1)
    I2 = s2 / pilot**5 / denom
    J1 = _RK / I2
    base = J1 / _N
    return (np.sign(base) * np.abs(base) ** 0.2).astype(np.float32)


def kernel(particles, weights=None, **_unused):
    from concourse.bass_utils import run_bass_kernel_spmd

    particles = np.ascontiguousarray(np.asarray(particles), dtype=np.float32)
    assert particles.shape == (_B, _N, _D), particles.shape

    nc = _get_nc()
    in_maps = [{"p": particles[c]} for c in range(_B)]
    res = run_bass_kernel_spmd(nc, in_maps, list(range(_B)))

    out = np.empty((_B, _D), np.float32)
    for c in range(_B):
        out[c] = finalize(res.results[c]["sout"], res.results[c]["varout"])
    return out


# revision 19
# speedup vs baseline: 1.4180x; 1.0099x over previous
"""Trainium2 Bass kernel for nn_BandwidthPredictorNNHall.

Math: for each batch b (8 of them, one per NeuronCore) with particles
x [n=1024, d=4]:
    pilot_d = 1.0592 * std(x_d, ddof=1) * n^(-1/8)
    q = x / pilot,   K_ij = exp(-0.5 * |q_i - q_j|^2)
    s2_d = sum_ij K_ij ((q_jd - q_id)^2 - 1)
    s3-based bandwidth2 is exactly 0 (odd under i<->j) and treated as 0.
With Mp = [1, p, p^2] (n x 9, RAW particle units), every sum needed for
s2 is an entry of V = Mp^T K Mp; the host applies ~30 scalar flops.

Device pipeline per core (vs the v1 kernel: triangle exp + transpose-free
reduction):
  - One contiguous 16KB input DMA [128, 8, 4]; particle order is
    irrelevant (all reductions are pair-permutation-invariant), so the
    fastest descriptor layout wins.
  - Stats (sum p, sum p^2) as two tiny PE accumulation groups; the var ->
    1/pilot^2 chain needs only a reciprocal (no sqrt => single {Exp,Copy}
    activation table).
  - Feature-major operands: one PE transpose gives T32 [32, 128] (raw,
    partition-sliced per row tile for lhsT); 8 PE transposes + a
    ScalarE/DVE split scaled copy give QTrs = (p * 1/pilot^2)^T [4, 1024]
    (rhs). G_ij = sum_d p_id * (s2_d p_jd) = q_i . q_j.
  - K'' = exp(G - r_i/2) computed ONLY for the upper-triangle blocks
    (row tile I covers columns j >= 128 I): 589k exps instead of 1M.
    K''_ij = K_ij e^{+r_j/2}; the column scale cancels later.
  - Reduction with no PT transposes: for each stored block B(I,J),
        psW[:,J,:] += matmul(lhsT=B, rhs=Mp[I])   (= B^T Mp[I], [128, 9])
    using 0.5*Mp for the diagonal I==J block. After group J closes,
        Vs += matmul(lhsT=W_J, rhs=MX[J]),  MX = Mp . e^{-r_j/2}
    which cancels the e^{+r_j/2} column scale exactly. By block symmetry
    of the true K, V = Vs + Vs^T (computed on the host in f64).
"""

import sys

sys.path.insert(0, "/opt/trn_rl_repo")

import numpy as np

_B, _N, _D = 8, 1024, 4
_P = 128
_NT = _N // _P  # 8 row tiles
_NM = 1 + 2 * _D  # 9 basis columns: [1, p, p^2]
_INV_SQRT_2PI = 1.0 / np.sqrt(2.0 * np.pi)
_RK = 0.282095
_FACT = 1.0592 * float(_N) ** (-1.0 / (4 + _D))

# row tile I covers columns [128*I, 1024); KT column offset per tile
_W = [_N - _P * i for i in range(_NT)]
_OFF = [0] * _NT
for _i in range(1, _NT):
    _OFF[_i] = _OFF[_i - 1] + _W[_i - 1]
_KTW = sum(_W)  # 4608

_NC = None  # compiled Bass module cache


def _build_kernel():
    import concourse.bass as bass  # noqa: F401
    import concourse.tile as tile
    from concourse import bacc, mybir
    from concourse.masks import make_identity

    f32 = mybir.dt.float32
    fr = mybir.dt.float32r
    Act = mybir.ActivationFunctionType
    Alu = mybir.AluOpType
    Ax = mybir.AxisListType

    nc = bacc.Bacc("TRN2", target_bir_lowering=False, debug=False, num_devices=_B)
    p_in = nc.dram_tensor("p", [_N, _D], f32, kind="ExternalInput")
    s_out = nc.dram_tensor("sout", [_NM, _NM], f32, kind="ExternalOutput")
    var_out = nc.dram_tensor("varout", [_D, 1], f32, kind="ExternalOutput")

    with tile.TileContext(nc) as tc:
        with (
            tc.tile_pool(name="singles", bufs=1) as singles,
            tc.tile_pool(name="psBig", bufs=2, space="PSUM") as psBig,
            tc.tile_pool(name="psW", bufs=1, space="PSUM") as psWp,
            tc.tile_pool(name="psV", bufs=1, space="PSUM") as psVp,
            tc.tile_pool(name="psLate", bufs=2, space="PSUM") as psLate,
        ):
            # ---- input DMA first: contiguous 16KB, particle (8i+r) ->
            # (partition i, slot r); any particle<->(tile,lane) bijection
            # is valid for the pairwise sums.
            mstat = singles.tile([_P, _NT, _D], f32, tag="mstat")
            nc.sync.dma_start(
                out=mstat, in_=p_in[:].rearrange("(i r) d -> i r d", i=_P)
            )

            # ---- constants (Pool/DVE, overlap the DMA wait)
            ident128 = singles.tile([_P, _P], f32, tag="identf")
            make_identity(nc, ident128)
            ones128 = singles.tile([_P, 1], f32, tag="ones128")
            nc.vector.memset(ones128, 1.0)
            half4 = singles.tile([_D, _P], f32, tag="half4")
            nc.vector.memset(half4, 0.5)
            mtall = singles.tile([_P, _NT, _NM], f32, tag="mtall")
            nc.vector.memset(mtall[:, :, 0:1], 1.0)
            # dummy Exp so the activation-table load runs during the DMA wait
            warm = singles.tile([1, 1], f32, tag="warm")
            nc.scalar.activation(out=warm, in_=ones128[0:1, 0:1], func=Act.Exp)

            # ---- squares
            msq = singles.tile([_P, _NT, _D], f32, tag="msq")
            nc.vector.tensor_mul(msq, mstat, mstat)

            # ---- stats on PE: psS[:,0] = sum p, psS[:,1] = sum p^2
            # (regions of the psV bank; all groups in this bank run
            # sequentially: psS x2, psbc, then Vs0..Vs7)
            psVm = psVp.tile([_P, 16], f32, tag="psvm")
            psS = psVm[0:_D, 9:11]
            for t in range(_NT):
                nc.tensor.matmul(
                    psS[:, 0:1], lhsT=mstat[:, t, :], rhs=ones128,
                    start=(t == 0), stop=(t == _NT - 1), skip_group_check=True,
                )
            for t in range(_NT):
                nc.tensor.matmul(
                    psS[:, 1:2], lhsT=msq[:, t, :], rhs=ones128,
                    start=(t == 0), stop=(t == _NT - 1), skip_group_check=True,
                )
            # feature-major rhs staging: 8 transposes into one 2-bank tile
            psQa = psLate.tile([_P, 512], f32, name="psQa", tag="psl")
            psQb = psLate.tile([_P, 512], f32, name="psQb", tag="psl")
            for c in range(_NT):
                dst = psQa if c < 4 else psQb
                nc.tensor.transpose(
                    dst[0:_D, (c % 4) * _P : (c % 4 + 1) * _P],
                    mstat[:, c, :], ident128,
                )

            # ---- var chain (DVE): s2col = 1/pilot^2 as a [4,1] column
            sums = singles.tile([_D, 2], f32, tag="sums")
            nc.vector.tensor_copy(sums, psS)
            t1 = singles.tile([_D, 1], f32, tag="t1")
            nc.vector.tensor_scalar(
                out=t1, in0=sums[:, 0:1], scalar1=sums[:, 0:1],
                scalar2=-1.0 / _N, op0=Alu.mult, op1=Alu.mult,
            )
            den = singles.tile([_D, 1], f32, tag="den")
            nc.vector.tensor_add(den, t1, sums[:, 1:2])  # (n-1) var
            var_t = singles.tile([_D, 1], f32, tag="var_t")
            nc.vector.tensor_scalar_mul(var_t, den, 1.0 / (_N - 1))
            nc.sync.dma_start(out=var_out[:], in_=var_t)
            denf = singles.tile([_D, 1], f32, tag="denf")
            nc.vector.tensor_scalar_mul(denf, den, _FACT * _FACT / (_N - 1))
            s2col = singles.tile([_D, 1], f32, tag="s2col")
            nc.vector.reciprocal(s2col, denf)  # 1/pilot^2

            # ---- raw rhs QTrr = p^T [4, 1024]: split the PSUM->SBUF copy
            # across ScalarE and DVE (no stats dependency -> early)
            QTrr = singles.tile([_D, _N], fr, tag="qtrr")
            nc.scalar.copy(QTrr[:, 0:512], psQa[0:_D, :])
            nc.vector.tensor_copy(QTrr[:, 512:_N], psQb[0:_D, :])

            # ---- nhall = -r/2 per particle: diag(s2) via one DVE op, one
            # rank-1 PE broadcast, then multiply/reduce; no row-form hops
            diag4 = singles.tile([_D, _D], f32, tag="diag4")
            nc.vector.tensor_scalar_mul(diag4, ident128[0:_D, 0:_D], s2col)
            psbc = psVm[:, 12:16]
            nc.tensor.matmul(
                psbc, lhsT=half4, rhs=diag4, start=True, stop=True,
                skip_group_check=True,
            )  # 0.5 * s2_d broadcast to all partitions
            scr = singles.tile([_P, _NT, _D], f32, tag="scr")
            nc.vector.tensor_mul(
                scr, msq, psbc.unsqueeze(1).broadcast_to((_P, _NT, _D))
            )
            nhall = singles.tile([_P, _NT], f32, tag="nhall")
            nc.vector.tensor_reduce(
                out=nhall, in_=scr, axis=Ax.X, op=Alu.add, negate=True
            )
            cneg = singles.tile([_P, _NT], f32, tag="cneg")
            nc.scalar.activation(out=cneg, in_=nhall, func=Act.Exp)

            # ---- scaled lhsT slices QTls[:, c, :] = (s2 * p)^T per row
            # tile, from SBUF QTrr on the otherwise-idle Pool engine; Mp
            # tiles [1 | p | p^2] and mthalf (diagonal blocks) too
            QTls = singles.tile([_D, _NT, _P], fr, tag="qtls")
            for c in range(2):
                nc.gpsimd.tensor_scalar_mul(
                    QTls[:, c, :], QTrr[:, c * _P : (c + 1) * _P], s2col
                )
            nc.gpsimd.tensor_copy(mtall[:, :, 1 : 1 + _D], mstat)
            nc.gpsimd.tensor_copy(mtall[:, :, 1 + _D : _NM], msq)
            mthalf = singles.tile([_P, _NT, _NM], f32, tag="mthalf")
            nc.gpsimd.tensor_scalar_mul(mthalf, mtall, 0.5)
            for c in range(2, _NT):
                nc.gpsimd.tensor_scalar_mul(
                    QTls[:, c, :], QTrr[:, c * _P : (c + 1) * _P], s2col
                )

            # ---- main stream: per row tile I (ascending), Gram chunks for
            # columns [128I, 1024) -> one exp -> per-block W matmuls; the
            # W group J closes at I==J, then its Vs contribution fires.
            KT = singles.tile([_P, _KTW], f32, tag="kt")
            psW_t = [
                psWp.tile([_P, _NM], f32, name=f"psw{j}", tag="psw")
                for j in range(_NT)
            ]
            psVs = psVm[0:_NM, 0:_NM]
            wsb = singles.tile([_P, _NT, _NM], f32, tag="wsb")
            psg_t = [None] * _NT
            psg_base = [0, 0, 0, 0, 512, 640, 768, 896]

            def g_chunks(i):
                cs = _P * i
                if cs < 512:
                    return [(cs, 512), (512, _N)]
                return [(cs, _N)]

            def emit_g(i):
                if i < 4:
                    psg = psBig.tile([_P, _N], f32, tag="psg")
                else:
                    psg = psLate.tile(
                        [_P, _N - psg_base[i]], f32, name=f"psl{i}", tag="psl"
                    )
                psg_t[i] = psg
                for a, b in g_chunks(i):
                    nc.tensor.matmul(
                        psg[:, a - psg_base[i] : b - psg_base[i]],
                        lhsT=QTls[:, i, :],
                        rhs=QTrr[:, a:b],
                        start=True, stop=True,
                    )

            def emit_w(j, i_lo=0, i_hi=None):
                # group J = blocks (I, J) for I <= J; groups must not
                # interleave within the PSUM bank (corruption), but a
                # group's own matmuls may be split across emission points
                if i_hi is None:
                    i_hi = j + 1
                for i in range(i_lo, i_hi):
                    rhs = mthalf[:, i, :] if i == j else mtall[:, i, :]
                    nc.tensor.matmul(
                        psW_t[j],
                        lhsT=KT[:, _OFF[i] + _P * (j - i) : _OFF[i] + _P * (j - i + 1)],
                        rhs=rhs,
                        start=(i == 0), stop=(i == j), skip_group_check=True,
                    )

            def emit_vs(j):
                # e^{-r_j/2} per partition cancels the K'' column scale
                nc.vector.tensor_scalar_mul(
                    wsb[:, j, :], psW_t[j], cneg[:, j : j + 1]
                )
                nc.tensor.matmul(
                    psVs, lhsT=wsb[:, j, :], rhs=mtall[:, j, :],
                    start=(j == 0), stop=(j == _NT - 1),
                )

            emit_g(0)
            emit_g(1)
            for i in range(_NT):
                cs = _P * i
                nc.scalar.activation(
                    out=KT[:, _OFF[i] : _OFF[i] + _W[i]],
                    in_=psg_t[i][:, cs - psg_base[i] : _N - psg_base[i]],
                    func=Act.Exp,
                    bias=nhall[:, i : i + 1],
                )
                if i + 2 < _NT:
                    emit_g(i + 2)
                if i < _NT - 1:
                    emit_w(i)
                    if i == _NT - 2:
                        # last group's I<=6 blocks are all ready now
                        emit_w(_NT - 1, 0, _NT - 1)
                    emit_vs(i)
                else:
                    emit_w(i, _NT - 1, _NT)
                    emit_vs(i)

            Vt = singles.tile([_NM, _NM], f32, tag="vt")
            nc.vector.tensor_copy(Vt, psVs)

            nc.sync.dma_start(out=s_out[:], in_=Vt)

    nc.compile()
    return nc


def _get_nc():
    global _NC
    if _NC is None:
        _NC = _build_kernel()
    return _NC


def finalize(S, var):
    """Host-side tail: S [9,9] (V = S + S^T, raw-p units), var [4] ->
    bandwidth [4]."""
    S = S.astype(np.float64)
    V = S + S.T
    var = var.astype(np.float64).reshape(_D)
    pilot = _FACT * np.sqrt(var)
    d = np.arange(_D)
    s2 = (
        (V[0, 5 + d] + V[5 + d, 0] - 2.0 * V[1 + d, 1 + d]) / pilot**2 - V[0, 0]
    ) * _INV_SQRT_2PI
    denom = _N * (_N - 1)
    I2 = s2 / pilot**5 / denom
    J1 = _RK / I2
    base = J1 / _N
    return (np.sign(base) * np.abs(base) ** 0.2).astype(np.float32)


def kernel(particles, weights=None, **_unused):
    from concourse.bass_utils import run_bass_kernel_spmd

    particles = np.ascontiguousarray(np.asarray(particles), dtype=np.float32)
    assert particles.shape == (_B, _N, _D), particles.shape

    nc = _get_nc()
    in_maps = [{"p": particles[c]} for c in range(_B)]
    res = run_bass_kernel_spmd(nc, in_maps, list(range(_B)))

    out = np.empty((_B, _D), np.float32)
    for c in range(_B):
        out[c] = finalize(res.results[c]["sout"], res.results[c]["varout"])
    return out
